# revision 15
# baseline (speedup 1.0000x reference)
"""Trainium2 Bass kernel for the Householder-chain problem.

Computes y = x @ Q.T where Q = M_0 @ M_1 @ ... @ M_{N-1} is a product of
N=514 Householder reflections M_i = I - 2 v_i v_i^T / (v_i^T v_i + eps)
over S=512 dims, and x is [65536, 512].

Since each M_i is symmetric, Q.T = M_{N-1} @ ... @ M_0 =: A, and the
product collapses via the compact-WY representation with natural column
order:  A = I - V T V^T  where V = [v_0 ... v_{N-1}] (S x N) and
T^{-1} = R = stril(V^T V) + diag((||v_i||^2 + eps)/2)   (lower triangular).

Sharding (per the hint: "replicate the small vectors/Q params on all
devices; shard x row-wise"): A is a tiny parameter transformation
(512x512, from the 1.25 MB `vectors` parameter), computed once on the
host in float64 (exact) and replicated to all 8 cores as bf16; x is
sharded row-wise, 8192 rows per core.

The device kernel is the memory-bound streaming matmul y = x @ A, all
bf16: x is transposed + cast to bf16 on the host (halves HBM read
traffic), y is written bf16 and upcast on the host (halves write
traffic).  End-to-end rel err ~2.9e-3 (bf16 rounding of x, A, y;
validated in numpy against the float64 reference), vs the 2e-2 gate.

Device timeline per core: the whole 8 MiB x shard streams in on the SP
DMA queue from t=0 (it fits in SBUF); A arrives on the Activation queue;
the PE then runs 256 back-to-back [128x128]x[128x512] bf16 matmuls
(~71 us, the measured PE streaming rate); PSUM->SBUF bf16 casts
alternate between the Vector and Scalar engines, and y tiles go out in
512-row batches on the Activation DMA queue, overlapped with compute.
"""

from contextlib import ExitStack

import numpy as np
import ml_dtypes

import bass_rust
import concourse.bass as bass
import concourse.mybir as mybir
import concourse.tile as tile
from concourse.bass_utils import run_bass_kernel_spmd
from concourse.vector_clock import ScopedClock

FP = mybir.dt.float32
BF = mybir.dt.bfloat16

S = 512           # feature dim
NV = 514          # number of householder vectors
B = 65536         # batch rows
NCORES = 8
BPC = B // NCORES  # 8192 rows per core
EPS = 1e-16
# main-loop x chunk widths (batch cols per chunk): a small leading chunk so
# the first matmul can start right after the fixed startup; big trailing
# chunks to minimize DMA descriptor count (the DMA queue processes ~135
# descriptors/us, one per partition per transfer).
CHUNKS = [512, 1024, 2560, 4096]
assert sum(CHUNKS) == BPC
# y DMA batch widths (batch cols per DMA), per s'-block pass: big batches
# early (4 KB descriptors), small ones at the end so the tail drains fast.
YBATCH = [2048, 2048, 2048, 1024, 512, 512]
assert sum(YBATCH) == BPC
WARMUP_MM = 8     # dummy matmuls to ramp the PE p-state during DMA


# ---------------------------------------------------------------------------
# walrus CTRL instructions accept at most 4 sem waits, and this Tile
# version puts the whole global-clock wait set on the single tail drain.
# Spread the waits over preceding SP nops (1 wait each, conservatively).
def _patched_drain_and_barrier(self, tick_clock, wait_clock):
    # Leave all the global-clock waits on the drain; _split_excess_waits
    # (run at the end of build_program) hoists the excess onto exactly
    # len(waits)-1 same-engine NOPs.
    drain_inst = self.nc.sync.drain()
    wait_clock.add_sem_waits(
        drain_inst.ins, ScopedClock({None: tick_clock.global_clock})
    )
    self.nc.all_engine_barrier()
    assert self.sems is not None
    popped = self.nc._tile_sem_poison_stack.pop()
    assert popped is self._sem_poison
    # Skip the end-of-program semaphore clear + second barrier (~5 us of
    # teardown): the startup sequence re-initializes every semaphore, so a
    # re-execution of the NEFF is unaffected.  Keep the allocator
    # bookkeeping (mirrors clear_and_free_semaphores minus the emitted
    # instructions) so pool release stays consistent.
    sem_nums = [s.num if hasattr(s, "num") else s
                for s in self.sems.allocated().values()]
    self.nc._state.prepend_free_semaphores(sem_nums)
    for poison_set in self.nc._tile_sem_poison_stack:
        poison_set.update(sem_nums)


tile.TileContext._drain_and_barrier = _patched_drain_and_barrier


def _split_excess_waits(nc, max_waits=1):
    """This walrus build accepts very few sem waits per instruction (a
    TensorTensor with 2 was rejected).  Hoist all but `max_waits` of each
    instruction's waits onto same-engine NOPs inserted right before it —
    engines execute in order, so semantics are unchanged."""
    idx = 0
    for fn in nc.m.functions:
        for bb in fn.blocks:
            new = []
            changed = False
            for inst in bb.instructions:
                si = inst.sync_info
                waits = list(si.on_wait) if si is not None and si.on_wait else []
                if len(waits) > max_waits:
                    changed = True
                    for w in waits[:-max_waits]:
                        idx += 1
                        nop = mybir.InstNoOp(
                            name=f"I-waitsplit-{idx}", engine=inst.engine)
                        nop.sync_info = bass_rust.SyncInfo(
                            on_wait=[w], on_update=[])
                        new.append(nop)
                    upd = list(si.on_update) if si.on_update else []
                    inst.sync_info = bass_rust.SyncInfo(
                        on_wait=waits[-max_waits:], on_update=upd)
                new.append(inst)
            if changed:
                bb.instructions = new
# ---------------------------------------------------------------------------


def build_program(trace_sim=False):
    nc = bass.Bass("TRN2")
    xt_d = nc.dram_tensor("xt", [S, BPC], BF, kind="ExternalInput")
    a_d = nc.dram_tensor("a", [S, S], BF, kind="ExternalInput")
    y_d = nc.dram_tensor("y", [S, BPC], BF, kind="ExternalOutput")

    with tile.TileContext(nc, trace_sim=trace_sim) as tc, ExitStack() as ctx:
        consts = ctx.enter_context(tc.tile_pool(name="consts", bufs=1))
        xbuf = ctx.enter_context(tc.tile_pool(name="xbuf", bufs=1))
        ypool = ctx.enter_context(tc.tile_pool(name="ypool", bufs=4))
        psum_y = ctx.enter_context(
            tc.tile_pool(name="psum_y", bufs=6, space="PSUM"))

        # x prefetch: the whole 8 MiB shard, SP HWDGE queue, from t=0.
        # bc_of[i] = (tiles, col offset within chunk) for batch-col step i
        # (512 cols per step).
        bc_of = []
        col0 = 0
        for c, cw in enumerate(CHUNKS):
            tiles = []
            for k in range(4):
                t = xbuf.tile([128, cw], BF, tag=f"xc{c}_{k}",
                              name=f"xc{c}_{k}")
                nc.sync.dma_start(
                    out=t, in_=xt_d[k * 128:(k + 1) * 128, col0:col0 + cw])
                tiles.append(t)
            for bt in range(cw // 512):
                bc_of.append((tiles, bt * 512))
            col0 += cw

        # A tiles on the Activation HWDGE queue.
        a_bf = []
        for k in range(4):
            t = consts.tile([128, S], BF, tag=f"a{k}", name=f"a{k}")
            nc.scalar.dma_start(out=t, in_=a_d[k * 128:(k + 1) * 128, :])
            a_bf.append(t)

        # PE p-state warmup: harmless matmuls while the DMAs land.
        zs = consts.tile([128, 128], BF, tag="zs", name="zs")
        nc.gpsimd.memset(zs, 0.0)
        zw = consts.tile([128, S], BF, tag="zw", name="zw")
        nc.gpsimd.memset(zw, 0.0)
        for i in range(WARMUP_MM):
            wp = psum_y.tile([128, S], FP, tag="y_ps", name=f"warmup{i}")
            nc.tensor.matmul(wp, lhsT=zs, rhs=zw, start=True, stop=True)

        # main loop: 4 passes, one per 128-wide s'-block of A, each
        # producing yT[st*128:(st+1)*128, :] across the whole batch.  The
        # output is written transposed (yT [S, BPC]) so every y DMA is
        # one contiguous multi-KB descriptor per partition.
        for st in range(4):
            ss = slice(st * 128, (st + 1) * 128)
            bc = 0
            for bw in YBATCH:
                nb = bw // 512
                yt = ypool.tile([128, YBATCH[0]], BF, tag="yt")
                for j in range(nb):
                    tiles, coff = bc_of[bc // 512]
                    y_ps = psum_y.tile([128, S], FP, tag="y_ps")
                    for k in range(4):
                        nc.tensor.matmul(
                            y_ps, lhsT=a_bf[k][:, ss],
                            rhs=tiles[k][:, coff:coff + 512],
                            start=(k == 0), stop=(k == 3))
                    dst = yt[:, j * S:(j + 1) * S]
                    if (bc // 512) % 2 == 0:
                        nc.vector.tensor_copy(dst, y_ps)
                    else:
                        nc.scalar.copy(dst, y_ps)
                    bc += 512
                nc.scalar.dma_start(out=y_d[ss, bc - bw:bc],
                                    in_=yt[:, 0:bw])
    _split_excess_waits(nc)
    return nc


_NC_CACHE = {}


def _get_nc():
    if "nc" not in _NC_CACHE:
        _NC_CACHE["nc"] = build_program()
    return _NC_CACHE["nc"]


def _compute_A(vectors):
    """Exact (float64) WY collapse of the Householder chain: A = Q^T."""
    v = np.asarray(vectors, dtype=np.float64)[..., 0]   # [514, 512]
    n, s = v.shape
    G = v @ v.T
    d = (np.sum(v * v, axis=1) + EPS) / 2.0
    R = np.tril(G, -1) + np.diag(d)
    X = np.linalg.inv(R)                                # T = R^{-1}
    A = np.eye(s) - v.T @ (X @ v)
    return A


def prepare_in_maps(x, vectors):
    x = np.asarray(x, dtype=np.float32)
    A = _compute_A(vectors).astype(ml_dtypes.bfloat16)  # [512, 512]
    xt = np.ascontiguousarray(x.T).astype(ml_dtypes.bfloat16)  # [512, 65536]
    in_maps = []
    for c in range(NCORES):
        in_maps.append({
            "xt": np.ascontiguousarray(xt[:, c * BPC:(c + 1) * BPC]),
            "a": A,
        })
    return in_maps


def gather_out(results):
    y = np.concatenate([r["y"] for r in results], axis=1)      # [512, B]
    return np.ascontiguousarray(y.T.astype(np.float32))


def kernel(x, vectors):
    nc = _get_nc()
    in_maps = prepare_in_maps(x, vectors)
    res = run_bass_kernel_spmd(nc, in_maps, list(range(NCORES)))
    return gather_out(res.results)


if __name__ == "__main__":
    rng = np.random.default_rng(0)
    x = rng.standard_normal((B, S)).astype(np.float32)
    v = rng.standard_normal((NV, S, 1)).astype(np.float32)
    v /= np.linalg.norm(v, axis=1, keepdims=True)
    y = kernel(x, v)
    print("y", y.shape, y.dtype, float(np.abs(y).max()))


# revision 16
# speedup vs baseline: 1.0377x; 1.0377x over previous
"""Trainium2 Bass kernel for the Householder-chain problem.

Computes y = x @ Q.T where Q = M_0 @ M_1 @ ... @ M_{N-1} is a product of
N=514 Householder reflections M_i = I - 2 v_i v_i^T / (v_i^T v_i + eps)
over S=512 dims, and x is [65536, 512].

Since each M_i is symmetric, Q.T = M_{N-1} @ ... @ M_0 =: A, and the
product collapses via the compact-WY representation with natural column
order:  A = I - V T V^T  where V = [v_0 ... v_{N-1}] (S x N) and
T^{-1} = R = stril(V^T V) + diag((||v_i||^2 + eps)/2)   (lower triangular).

Sharding (per the hint: "replicate the small vectors/Q params on all
devices; shard x row-wise"): A is a tiny parameter transformation
(512x512, from the 1.25 MB `vectors` parameter), computed once on the
host in float64 (exact) and replicated to all 8 cores as bf16; x is
sharded row-wise, 8192 rows per core.

The device kernel is the memory-bound streaming matmul y = x @ A, all
bf16: x is transposed + cast to bf16 on the host (halves HBM read
traffic), y is written bf16 and upcast on the host (halves write
traffic).  End-to-end rel err ~2.9e-3 (bf16 rounding of x, A, y;
validated in numpy against the float64 reference), vs the 2e-2 gate.

Device timeline per core: the whole 8 MiB x shard streams in on the SP
DMA queue from t=0 (it fits in SBUF); A arrives on the Activation queue;
the PE then runs 256 back-to-back [128x128]x[128x512] bf16 matmuls
(~71 us, the measured PE streaming rate); PSUM->SBUF bf16 casts
alternate between the Vector and Scalar engines, and y tiles go out in
512-row batches on the Activation DMA queue, overlapped with compute.
"""

from contextlib import ExitStack

import numpy as np
import ml_dtypes

import bass_rust
import concourse.bass as bass
import concourse.mybir as mybir
import concourse.tile as tile
from concourse.bass_utils import run_bass_kernel_spmd
from concourse.vector_clock import ScopedClock

FP = mybir.dt.float32
BF = mybir.dt.bfloat16

S = 512           # feature dim
NV = 514          # number of householder vectors
B = 65536         # batch rows
NCORES = 8
BPC = B // NCORES  # 8192 rows per core
EPS = 1e-16
# main-loop x chunk widths (batch cols per chunk): a small leading chunk so
# the first matmul can start right after the fixed startup; big trailing
# chunks to minimize DMA descriptor count (the DMA queue processes ~135
# descriptors/us, one per partition per transfer).
CHUNKS = [512, 1024, 2560, 4096]
assert sum(CHUNKS) == BPC
# y DMA batch widths (batch cols per DMA), per s'-block pass: big batches
# early (4 KB descriptors), small ones at the end so the tail drains fast.
YBATCH = [2048, 2048, 2048, 1024, 512, 512]
assert sum(YBATCH) == BPC
WARMUP_MM = 8     # dummy matmuls to ramp the PE p-state during DMA


# ---------------------------------------------------------------------------
# walrus CTRL instructions accept at most 4 sem waits, and this Tile
# version puts the whole global-clock wait set on the single tail drain.
# Spread the waits over preceding SP nops (1 wait each, conservatively).
def _patched_drain_and_barrier(self, tick_clock, wait_clock):
    # Leave all the global-clock waits on the drain; _split_excess_waits
    # (run at the end of build_program) hoists the excess onto exactly
    # len(waits)-1 same-engine NOPs.
    drain_inst = self.nc.sync.drain()
    wait_clock.add_sem_waits(
        drain_inst.ins, ScopedClock({None: tick_clock.global_clock})
    )
    self.nc.all_engine_barrier()
    assert self.sems is not None
    popped = self.nc._tile_sem_poison_stack.pop()
    assert popped is self._sem_poison
    # Skip the end-of-program semaphore clear + second barrier (~5 us of
    # teardown): the startup sequence re-initializes every semaphore, so a
    # re-execution of the NEFF is unaffected.  Keep the allocator
    # bookkeeping (mirrors clear_and_free_semaphores minus the emitted
    # instructions) so pool release stays consistent.
    sem_nums = [s.num if hasattr(s, "num") else s
                for s in self.sems.allocated().values()]
    self.nc._state.prepend_free_semaphores(sem_nums)
    for poison_set in self.nc._tile_sem_poison_stack:
        poison_set.update(sem_nums)


tile.TileContext._drain_and_barrier = _patched_drain_and_barrier


def _split_excess_waits(nc, max_waits=1):
    """This walrus build accepts very few sem waits per instruction (a
    TensorTensor with 2 was rejected).  Hoist all but `max_waits` of each
    instruction's waits onto same-engine NOPs inserted right before it —
    engines execute in order, so semantics are unchanged."""
    idx = 0
    for fn in nc.m.functions:
        for bb in fn.blocks:
            new = []
            changed = False
            for inst in bb.instructions:
                si = inst.sync_info
                waits = list(si.on_wait) if si is not None and si.on_wait else []
                if len(waits) > max_waits:
                    changed = True
                    for w in waits[:-max_waits]:
                        idx += 1
                        nop = mybir.InstNoOp(
                            name=f"I-waitsplit-{idx}", engine=inst.engine)
                        nop.sync_info = bass_rust.SyncInfo(
                            on_wait=[w], on_update=[])
                        new.append(nop)
                    upd = list(si.on_update) if si.on_update else []
                    inst.sync_info = bass_rust.SyncInfo(
                        on_wait=waits[-max_waits:], on_update=upd)
                new.append(inst)
            if changed:
                bb.instructions = new
# ---------------------------------------------------------------------------


def build_program(trace_sim=False):
    nc = bass.Bass("TRN2")
    xt_d = nc.dram_tensor("xt", [S, BPC], BF, kind="ExternalInput")
    a_d = nc.dram_tensor("a", [S, S], BF, kind="ExternalInput")
    y_d = nc.dram_tensor("y", [S, BPC], BF, kind="ExternalOutput")

    with tile.TileContext(nc, trace_sim=trace_sim) as tc, ExitStack() as ctx:
        consts = ctx.enter_context(tc.tile_pool(name="consts", bufs=1))
        xbuf = ctx.enter_context(tc.tile_pool(name="xbuf", bufs=1))
        ypool = ctx.enter_context(tc.tile_pool(name="ypool", bufs=4))
        psum_y = ctx.enter_context(
            tc.tile_pool(name="psum_y", bufs=6, space="PSUM"))

        # x prefetch: the whole 8 MiB shard, SP HWDGE queue, from t=0.
        # bc_of[i] = (tiles, col offset within chunk) for batch-col step i
        # (512 cols per step).
        bc_of = []
        col0 = 0
        for c, cw in enumerate(CHUNKS):
            tiles = []
            for k in range(4):
                t = xbuf.tile([128, cw], BF, tag=f"xc{c}_{k}",
                              name=f"xc{c}_{k}")
                nc.sync.dma_start(
                    out=t, in_=xt_d[k * 128:(k + 1) * 128, col0:col0 + cw])
                tiles.append(t)
            for bt in range(cw // 512):
                bc_of.append((tiles, bt * 512))
            col0 += cw

        # A tiles on the Activation HWDGE queue.
        a_bf = []
        for k in range(4):
            t = consts.tile([128, S], BF, tag=f"a{k}", name=f"a{k}")
            nc.scalar.dma_start(out=t, in_=a_d[k * 128:(k + 1) * 128, :])
            a_bf.append(t)

        # PE p-state warmup: harmless matmuls while the DMAs land.
        zs = consts.tile([128, 128], BF, tag="zs", name="zs")
        nc.gpsimd.memset(zs, 0.0)
        zw = consts.tile([128, S], BF, tag="zw", name="zw")
        nc.gpsimd.memset(zw, 0.0)
        for i in range(WARMUP_MM):
            wp = psum_y.tile([128, S], FP, tag="y_ps", name=f"warmup{i}")
            nc.tensor.matmul(wp, lhsT=zs, rhs=zw, start=True, stop=True)

        # main loop over batch-column groups; within a group, one pass per
        # 128-wide s'-block of A, writing yT[st-block, group-cols].  The
        # output is transposed (yT [S, BPC]) so every y DMA is one
        # contiguous multi-KB descriptor per partition, and group-outer
        # order keeps x consumption behind the x DMA stream.
        g0 = 0
        eng = 0
        for bw in YBATCH:
            nb = bw // 512
            for st in range(4):
                ss = slice(st * 128, (st + 1) * 128)
                yt = ypool.tile([128, YBATCH[0]], BF, tag=f"yt{st}", bufs=2)
                for j in range(nb):
                    bc = g0 + j * 512
                    tiles, coff = bc_of[bc // 512]
                    y_ps = psum_y.tile([128, S], FP, tag="y_ps")
                    for k in range(4):
                        nc.tensor.matmul(
                            y_ps, lhsT=a_bf[k][:, ss],
                            rhs=tiles[k][:, coff:coff + 512],
                            start=(k == 0), stop=(k == 3))
                    dst = yt[:, j * S:(j + 1) * S]
                    eng += 1
                    if eng % 2 == 0:
                        nc.vector.tensor_copy(dst, y_ps)
                    else:
                        nc.scalar.copy(dst, y_ps)
                nc.scalar.dma_start(out=y_d[ss, g0:g0 + bw],
                                    in_=yt[:, 0:bw])
            g0 += bw
    _split_excess_waits(nc)
    return nc


_NC_CACHE = {}


def _get_nc():
    if "nc" not in _NC_CACHE:
        _NC_CACHE["nc"] = build_program()
    return _NC_CACHE["nc"]


def _compute_A(vectors):
    """Exact (float64) WY collapse of the Householder chain: A = Q^T."""
    v = np.asarray(vectors, dtype=np.float64)[..., 0]   # [514, 512]
    n, s = v.shape
    G = v @ v.T
    d = (np.sum(v * v, axis=1) + EPS) / 2.0
    R = np.tril(G, -1) + np.diag(d)
    X = np.linalg.inv(R)                                # T = R^{-1}
    A = np.eye(s) - v.T @ (X @ v)
    return A


def prepare_in_maps(x, vectors):
    x = np.asarray(x, dtype=np.float32)
    A = _compute_A(vectors).astype(ml_dtypes.bfloat16)  # [512, 512]
    xt = np.ascontiguousarray(x.T).astype(ml_dtypes.bfloat16)  # [512, 65536]
    in_maps = []
    for c in range(NCORES):
        in_maps.append({
            "xt": np.ascontiguousarray(xt[:, c * BPC:(c + 1) * BPC]),
            "a": A,
        })
    return in_maps


def gather_out(results):
    y = np.concatenate([r["y"] for r in results], axis=1)      # [512, B]
    return np.ascontiguousarray(y.T.astype(np.float32))


def kernel(x, vectors):
    nc = _get_nc()
    in_maps = prepare_in_maps(x, vectors)
    res = run_bass_kernel_spmd(nc, in_maps, list(range(NCORES)))
    return gather_out(res.results)


if __name__ == "__main__":
    rng = np.random.default_rng(0)
    x = rng.standard_normal((B, S)).astype(np.float32)
    v = rng.standard_normal((NV, S, 1)).astype(np.float32)
    v /= np.linalg.norm(v, axis=1, keepdims=True)
    y = kernel(x, v)
    print("y", y.shape, y.dtype, float(np.abs(y).max()))


# revision 17
# speedup vs baseline: 1.1501x; 1.1083x over previous
"""Trainium2 Bass kernel for the Householder-chain problem.

Computes y = x @ Q.T where Q = M_0 @ M_1 @ ... @ M_{N-1} is a product of
N=514 Householder reflections M_i = I - 2 v_i v_i^T / (v_i^T v_i + eps)
over S=512 dims, and x is [65536, 512].

Since each M_i is symmetric, Q.T = M_{N-1} @ ... @ M_0 =: A, and the
product collapses via the compact-WY representation with natural column
order:  A = I - V T V^T  where V = [v_0 ... v_{N-1}] (S x N) and
T^{-1} = R = stril(V^T V) + diag((||v_i||^2 + eps)/2)   (lower triangular).

Sharding (per the hint: "replicate the small vectors/Q params on all
devices; shard x row-wise"): A is a tiny parameter transformation
(512x512, from the 1.25 MB `vectors` parameter), computed once on the
host in float64 (exact) and replicated to all 8 cores as bf16; x is
sharded row-wise, 8192 rows per core.

The device kernel is the memory-bound streaming matmul y = x @ A, all
bf16: x is transposed + cast to bf16 on the host (halves HBM read
traffic), y is written bf16 and upcast on the host (halves write
traffic).  End-to-end rel err ~2.9e-3 (bf16 rounding of x, A, y;
validated in numpy against the float64 reference), vs the 2e-2 gate.

Device timeline per core: the whole 8 MiB x shard streams in on the SP
DMA queue from t=0 (it fits in SBUF); A arrives on the Activation queue;
the PE then runs 256 back-to-back [128x128]x[128x512] bf16 matmuls
(~71 us, the measured PE streaming rate); PSUM->SBUF bf16 casts
alternate between the Vector and Scalar engines, and y tiles go out in
512-row batches on the Activation DMA queue, overlapped with compute.
"""

from contextlib import ExitStack

import numpy as np
import ml_dtypes

import bass_rust
import concourse.bass as bass
import concourse.mybir as mybir
import concourse.tile as tile
from concourse.bass_utils import run_bass_kernel_spmd
from concourse.vector_clock import ScopedClock

FP = mybir.dt.float32
BF = mybir.dt.bfloat16

S = 512           # feature dim
NV = 514          # number of householder vectors
B = 65536         # batch rows
NCORES = 8
BPC = B // NCORES  # 8192 rows per core
EPS = 1e-16
# main-loop x chunk widths (batch cols per chunk): a small leading chunk so
# the first matmul can start right after the fixed startup; big trailing
# chunks to minimize DMA descriptor count (the DMA queue processes ~135
# descriptors/us, one per partition per transfer).
CHUNKS = [512, 512] + [1024] * 7
assert sum(CHUNKS) == BPC
# y DMA batch widths (batch cols per DMA), per s'-block pass: big batches
# early (4 KB descriptors), small ones at the end so the tail drains fast.
YBATCH = [2048, 2048, 2048, 1024, 512, 512]
assert sum(YBATCH) == BPC
WARMUP_MM = 8     # dummy matmuls to ramp the PE p-state during DMA


# ---------------------------------------------------------------------------
# walrus CTRL instructions accept at most 4 sem waits, and this Tile
# version puts the whole global-clock wait set on the single tail drain.
# Spread the waits over preceding SP nops (1 wait each, conservatively).
def _patched_drain_and_barrier(self, tick_clock, wait_clock):
    # Leave all the global-clock waits on the drain; _split_excess_waits
    # (run at the end of build_program) hoists the excess onto exactly
    # len(waits)-1 same-engine NOPs.
    drain_inst = self.nc.sync.drain()
    wait_clock.add_sem_waits(
        drain_inst.ins, ScopedClock({None: tick_clock.global_clock})
    )
    self.nc.all_engine_barrier()
    assert self.sems is not None
    popped = self.nc._tile_sem_poison_stack.pop()
    assert popped is self._sem_poison
    # Skip the end-of-program semaphore clear + second barrier (~5 us of
    # teardown): the startup sequence re-initializes every semaphore, so a
    # re-execution of the NEFF is unaffected.  Keep the allocator
    # bookkeeping (mirrors clear_and_free_semaphores minus the emitted
    # instructions) so pool release stays consistent.
    sem_nums = [s.num if hasattr(s, "num") else s
                for s in self.sems.allocated().values()]
    self.nc._state.prepend_free_semaphores(sem_nums)
    for poison_set in self.nc._tile_sem_poison_stack:
        poison_set.update(sem_nums)


tile.TileContext._drain_and_barrier = _patched_drain_and_barrier


def _split_excess_waits(nc, max_waits=1):
    """This walrus build accepts very few sem waits per instruction (a
    TensorTensor with 2 was rejected).  Hoist all but `max_waits` of each
    instruction's waits onto same-engine NOPs inserted right before it —
    engines execute in order, so semantics are unchanged."""
    idx = 0
    for fn in nc.m.functions:
        for bb in fn.blocks:
            new = []
            changed = False
            for inst in bb.instructions:
                si = inst.sync_info
                waits = list(si.on_wait) if si is not None and si.on_wait else []
                if len(waits) > max_waits:
                    changed = True
                    for w in waits[:-max_waits]:
                        idx += 1
                        nop = mybir.InstNoOp(
                            name=f"I-waitsplit-{idx}", engine=inst.engine)
                        nop.sync_info = bass_rust.SyncInfo(
                            on_wait=[w], on_update=[])
                        new.append(nop)
                    upd = list(si.on_update) if si.on_update else []
                    inst.sync_info = bass_rust.SyncInfo(
                        on_wait=waits[-max_waits:], on_update=upd)
                new.append(inst)
            if changed:
                bb.instructions = new
# ---------------------------------------------------------------------------


def build_program(trace_sim=False):
    nc = bass.Bass("TRN2")
    xt_d = nc.dram_tensor("xt", [S, BPC], BF, kind="ExternalInput")
    a_d = nc.dram_tensor("a", [S, S], BF, kind="ExternalInput")
    y_d = nc.dram_tensor("y", [S, BPC], BF, kind="ExternalOutput")

    with tile.TileContext(nc, trace_sim=trace_sim) as tc, ExitStack() as ctx:
        consts = ctx.enter_context(tc.tile_pool(name="consts", bufs=1))
        xbuf = ctx.enter_context(tc.tile_pool(name="xbuf", bufs=1))
        ypool = ctx.enter_context(tc.tile_pool(name="ypool", bufs=4))
        psum_y = ctx.enter_context(
            tc.tile_pool(name="psum_y", bufs=6, space="PSUM"))

        # x prefetch: the whole 8 MiB shard, SP HWDGE queue, from t=0.
        # bc_of[i] = (tiles, col offset within chunk) for batch-col step i
        # (512 cols per step).
        bc_of = []
        col0 = 0
        for c, cw in enumerate(CHUNKS):
            tiles = []
            for k in range(4):
                t = xbuf.tile([128, cw], BF, tag=f"xc{c}_{k}",
                              name=f"xc{c}_{k}")
                nc.sync.dma_start(
                    out=t, in_=xt_d[k * 128:(k + 1) * 128, col0:col0 + cw])
                tiles.append(t)
            for bt in range(cw // 512):
                bc_of.append((tiles, bt * 512))
            col0 += cw

        # A tiles on the Activation HWDGE queue.
        a_bf = []
        for k in range(4):
            t = consts.tile([128, S], BF, tag=f"a{k}", name=f"a{k}")
            nc.scalar.dma_start(out=t, in_=a_d[k * 128:(k + 1) * 128, :])
            a_bf.append(t)

        # PE p-state warmup: harmless matmuls while the DMAs land.
        zs = consts.tile([128, 128], BF, tag="zs", name="zs")
        nc.gpsimd.memset(zs, 0.0)
        zw = consts.tile([128, S], BF, tag="zw", name="zw")
        nc.gpsimd.memset(zw, 0.0)
        for i in range(WARMUP_MM):
            wp = psum_y.tile([128, S], FP, tag="y_ps", name=f"warmup{i}")
            nc.tensor.matmul(wp, lhsT=zs, rhs=zw, start=True, stop=True)

        # main loop over batch-column groups; within a group, one pass per
        # 128-wide s'-block of A, writing yT[st-block, group-cols].  The
        # output is transposed (yT [S, BPC]) so every y DMA is one
        # contiguous multi-KB descriptor per partition, and group-outer
        # order keeps x consumption behind the x DMA stream.
        g0 = 0
        eng = 0
        for bw in YBATCH:
            nb = bw // 512
            for st in range(4):
                ss = slice(st * 128, (st + 1) * 128)
                yt = ypool.tile([128, YBATCH[0]], BF, tag=f"yt{st}", bufs=2)
                for j in range(nb):
                    bc = g0 + j * 512
                    tiles, coff = bc_of[bc // 512]
                    y_ps = psum_y.tile([128, S], FP, tag="y_ps")
                    for k in range(4):
                        nc.tensor.matmul(
                            y_ps, lhsT=a_bf[k][:, ss],
                            rhs=tiles[k][:, coff:coff + 512],
                            start=(k == 0), stop=(k == 3))
                    dst = yt[:, j * S:(j + 1) * S]
                    eng += 1
                    if eng % 2 == 0:
                        nc.vector.tensor_copy(dst, y_ps)
                    else:
                        nc.scalar.copy(dst, y_ps)
                nc.scalar.dma_start(out=y_d[ss, g0:g0 + bw],
                                    in_=yt[:, 0:bw])
            g0 += bw
    _split_excess_waits(nc)
    return nc


_NC_CACHE = {}


def _get_nc():
    if "nc" not in _NC_CACHE:
        _NC_CACHE["nc"] = build_program()
    return _NC_CACHE["nc"]


def _compute_A(vectors):
    """Exact (float64) WY collapse of the Householder chain: A = Q^T."""
    v = np.asarray(vectors, dtype=np.float64)[..., 0]   # [514, 512]
    n, s = v.shape
    G = v @ v.T
    d = (np.sum(v * v, axis=1) + EPS) / 2.0
    R = np.tril(G, -1) + np.diag(d)
    X = np.linalg.inv(R)                                # T = R^{-1}
    A = np.eye(s) - v.T @ (X @ v)
    return A


def prepare_in_maps(x, vectors):
    x = np.asarray(x, dtype=np.float32)
    A = _compute_A(vectors).astype(ml_dtypes.bfloat16)  # [512, 512]
    xt = np.ascontiguousarray(x.T).astype(ml_dtypes.bfloat16)  # [512, 65536]
    in_maps = []
    for c in range(NCORES):
        in_maps.append({
            "xt": np.ascontiguousarray(xt[:, c * BPC:(c + 1) * BPC]),
            "a": A,
        })
    return in_maps


def gather_out(results):
    y = np.concatenate([r["y"] for r in results], axis=1)      # [512, B]
    return np.ascontiguousarray(y.T.astype(np.float32))


def kernel(x, vectors):
    nc = _get_nc()
    in_maps = prepare_in_maps(x, vectors)
    res = run_bass_kernel_spmd(nc, in_maps, list(range(NCORES)))
    return gather_out(res.results)


if __name__ == "__main__":
    rng = np.random.default_rng(0)
    x = rng.standard_normal((B, S)).astype(np.float32)
    v = rng.standard_normal((NV, S, 1)).astype(np.float32)
    v /= np.linalg.norm(v, axis=1, keepdims=True)
    y = kernel(x, v)
    print("y", y.shape, y.dtype, float(np.abs(y).max()))


# revision 19
# speedup vs baseline: 1.1759x; 1.0224x over previous
"""Trainium2 Bass kernel for the Householder-chain problem.

Computes y = x @ Q.T where Q = M_0 @ M_1 @ ... @ M_{N-1} is a product of
N=514 Householder reflections M_i = I - 2 v_i v_i^T / (v_i^T v_i + eps)
over S=512 dims, and x is [65536, 512].

Since each M_i is symmetric, Q.T = M_{N-1} @ ... @ M_0 =: A, and the
product collapses via the compact-WY representation with natural column
order:  A = I - V T V^T  where V = [v_0 ... v_{N-1}] (S x N) and
T^{-1} = R = stril(V^T V) + diag((||v_i||^2 + eps)/2)   (lower triangular).

Sharding (per the hint: "replicate the small vectors/Q params on all
devices; shard x row-wise"): A is a tiny parameter transformation
(512x512, from the 1.25 MB `vectors` parameter), computed once on the
host in float64 (exact) and replicated to all 8 cores as bf16; x is
sharded row-wise, 8192 rows per core.

The device kernel is the memory-bound streaming matmul y = x @ A, all
bf16: x is transposed + cast to bf16 on the host (halves HBM read
traffic), y is written bf16 and upcast on the host (halves write
traffic).  End-to-end rel err ~2.9e-3 (bf16 rounding of x, A, y;
validated in numpy against the float64 reference), vs the 2e-2 gate.

Device timeline per core: the whole 8 MiB x shard streams in on the SP
DMA queue from t=0 (it fits in SBUF); A arrives on the Activation queue;
the PE then runs 256 back-to-back [128x128]x[128x512] bf16 matmuls
(~71 us, the measured PE streaming rate); PSUM->SBUF bf16 casts
alternate between the Vector and Scalar engines, and y tiles go out in
512-row batches on the Activation DMA queue, overlapped with compute.
"""

from contextlib import ExitStack

import numpy as np
import ml_dtypes

import bass_rust
import concourse.bass as bass
import concourse.mybir as mybir
import concourse.tile as tile
from concourse.bass_utils import run_bass_kernel_spmd
from concourse.vector_clock import ScopedClock

FP = mybir.dt.float32
BF = mybir.dt.bfloat16

S = 512           # feature dim
NV = 514          # number of householder vectors
B = 65536         # batch rows
NCORES = 8
BPC = B // NCORES  # 8192 rows per core
EPS = 1e-16
# main-loop x chunk widths (batch cols per chunk): a small leading chunk so
# the first matmul can start right after the fixed startup; big trailing
# chunks to minimize DMA descriptor count (the DMA queue processes ~135
# descriptors/us, one per partition per transfer).
CHUNKS = [512, 512] + [1024] * 7
assert sum(CHUNKS) == BPC
# y DMA batch widths (batch cols per DMA), per s'-block pass: big batches
# early (4 KB descriptors), small ones at the end so the tail drains fast.
YBATCH = [1024, 1024, 2048, 2048, 1024, 512, 512]
assert sum(YBATCH) == BPC
WARMUP_MM = 8     # dummy matmuls to ramp the PE p-state during DMA


# ---------------------------------------------------------------------------
# walrus CTRL instructions accept at most 4 sem waits, and this Tile
# version puts the whole global-clock wait set on the single tail drain.
# Spread the waits over preceding SP nops (1 wait each, conservatively).
def _patched_drain_and_barrier(self, tick_clock, wait_clock):
    # Leave all the global-clock waits on the drain; _split_excess_waits
    # (run at the end of build_program) hoists the excess onto exactly
    # len(waits)-1 same-engine NOPs.
    drain_inst = self.nc.sync.drain()
    wait_clock.add_sem_waits(
        drain_inst.ins, ScopedClock({None: tick_clock.global_clock})
    )
    self.nc.all_engine_barrier()
    assert self.sems is not None
    popped = self.nc._tile_sem_poison_stack.pop()
    assert popped is self._sem_poison
    # Skip the end-of-program semaphore clear + second barrier (~5 us of
    # teardown): the startup sequence re-initializes every semaphore, so a
    # re-execution of the NEFF is unaffected.  Keep the allocator
    # bookkeeping (mirrors clear_and_free_semaphores minus the emitted
    # instructions) so pool release stays consistent.
    sem_nums = [s.num if hasattr(s, "num") else s
                for s in self.sems.allocated().values()]
    self.nc._state.prepend_free_semaphores(sem_nums)
    for poison_set in self.nc._tile_sem_poison_stack:
        poison_set.update(sem_nums)


tile.TileContext._drain_and_barrier = _patched_drain_and_barrier


def _split_excess_waits(nc, max_waits=1):
    """This walrus build accepts very few sem waits per instruction (a
    TensorTensor with 2 was rejected).  Hoist all but `max_waits` of each
    instruction's waits onto same-engine NOPs inserted right before it —
    engines execute in order, so semantics are unchanged."""
    idx = 0
    for fn in nc.m.functions:
        for bb in fn.blocks:
            new = []
            changed = False
            for inst in bb.instructions:
                si = inst.sync_info
                waits = list(si.on_wait) if si is not None and si.on_wait else []
                if len(waits) > max_waits:
                    changed = True
                    for w in waits[:-max_waits]:
                        idx += 1
                        nop = mybir.InstNoOp(
                            name=f"I-waitsplit-{idx}", engine=inst.engine)
                        nop.sync_info = bass_rust.SyncInfo(
                            on_wait=[w], on_update=[])
                        new.append(nop)
                    upd = list(si.on_update) if si.on_update else []
                    inst.sync_info = bass_rust.SyncInfo(
                        on_wait=waits[-max_waits:], on_update=upd)
                new.append(inst)
            if changed:
                bb.instructions = new
# ---------------------------------------------------------------------------


def build_program(trace_sim=False):
    nc = bass.Bass("TRN2")
    xt_d = nc.dram_tensor("xt", [S, BPC], BF, kind="ExternalInput")
    a_d = nc.dram_tensor("a", [S, S], BF, kind="ExternalInput")
    y_d = nc.dram_tensor("y", [S, BPC], BF, kind="ExternalOutput")

    with tile.TileContext(nc, trace_sim=trace_sim) as tc, ExitStack() as ctx:
        consts = ctx.enter_context(tc.tile_pool(name="consts", bufs=1))
        xbuf = ctx.enter_context(tc.tile_pool(name="xbuf", bufs=1))
        ypool = ctx.enter_context(tc.tile_pool(name="ypool", bufs=4))
        psum_y = ctx.enter_context(
            tc.tile_pool(name="psum_y", bufs=6, space="PSUM"))

        # x prefetch: the whole 8 MiB shard, SP HWDGE queue, from t=0.
        # bc_of[i] = (tiles, col offset within chunk) for batch-col step i
        # (512 cols per step).
        bc_of = []
        col0 = 0
        for c, cw in enumerate(CHUNKS):
            tiles = []
            for k in range(4):
                t = xbuf.tile([128, cw], BF, tag=f"xc{c}_{k}",
                              name=f"xc{c}_{k}")
                nc.sync.dma_start(
                    out=t, in_=xt_d[k * 128:(k + 1) * 128, col0:col0 + cw])
                tiles.append(t)
            for bt in range(cw // 512):
                bc_of.append((tiles, bt * 512))
            col0 += cw

        # A tiles on the Activation HWDGE queue.
        a_bf = []
        for k in range(4):
            t = consts.tile([128, S], BF, tag=f"a{k}", name=f"a{k}")
            nc.scalar.dma_start(out=t, in_=a_d[k * 128:(k + 1) * 128, :])
            a_bf.append(t)

        # PE p-state warmup: harmless matmuls while the DMAs land.
        zs = consts.tile([128, 128], BF, tag="zs", name="zs")
        nc.gpsimd.memset(zs, 0.0)
        zw = consts.tile([128, S], BF, tag="zw", name="zw")
        nc.gpsimd.memset(zw, 0.0)
        for i in range(WARMUP_MM):
            wp = psum_y.tile([128, S], FP, tag="y_ps", name=f"warmup{i}")
            nc.tensor.matmul(wp, lhsT=zs, rhs=zw, start=True, stop=True)

        # main loop over batch-column groups; within a group, one pass per
        # 128-wide s'-block of A, writing yT[st-block, group-cols].  The
        # output is transposed (yT [S, BPC]) so every y DMA is one
        # contiguous multi-KB descriptor per partition, and group-outer
        # order keeps x consumption behind the x DMA stream.
        g0 = 0
        eng = 0
        for bw in YBATCH:
            nb = bw // 512
            for st in range(4):
                ss = slice(st * 128, (st + 1) * 128)
                yt = ypool.tile([128, max(YBATCH)], BF, tag=f"yt{st}", bufs=2)
                for j in range(nb):
                    bc = g0 + j * 512
                    tiles, coff = bc_of[bc // 512]
                    y_ps = psum_y.tile([128, S], FP, tag="y_ps")
                    for k in range(4):
                        nc.tensor.matmul(
                            y_ps, lhsT=a_bf[k][:, ss],
                            rhs=tiles[k][:, coff:coff + 512],
                            start=(k == 0), stop=(k == 3))
                    dst = yt[:, j * S:(j + 1) * S]
                    eng += 1
                    if eng % 2 == 0:
                        nc.vector.tensor_copy(dst, y_ps)
                    else:
                        nc.scalar.copy(dst, y_ps)
                # the last two (small) groups alternate queues: the SP
                # queue's x stream is long done, so splitting the final
                # DMAs across both queues halves the drain tail.
                if bw <= 512 and st % 2 == 0:
                    nc.sync.dma_start(out=y_d[ss, g0:g0 + bw],
                                      in_=yt[:, 0:bw])
                else:
                    nc.scalar.dma_start(out=y_d[ss, g0:g0 + bw],
                                        in_=yt[:, 0:bw])
            g0 += bw
    _split_excess_waits(nc)
    return nc


_NC_CACHE = {}


def _get_nc():
    if "nc" not in _NC_CACHE:
        _NC_CACHE["nc"] = build_program()
    return _NC_CACHE["nc"]


def _compute_A(vectors):
    """Exact (float64) WY collapse of the Householder chain: A = Q^T."""
    v = np.asarray(vectors, dtype=np.float64)[..., 0]   # [514, 512]
    n, s = v.shape
    G = v @ v.T
    d = (np.sum(v * v, axis=1) + EPS) / 2.0
    R = np.tril(G, -1) + np.diag(d)
    X = np.linalg.inv(R)                                # T = R^{-1}
    A = np.eye(s) - v.T @ (X @ v)
    return A


def prepare_in_maps(x, vectors):
    x = np.asarray(x, dtype=np.float32)
    A = _compute_A(vectors).astype(ml_dtypes.bfloat16)  # [512, 512]
    xt = np.ascontiguousarray(x.T).astype(ml_dtypes.bfloat16)  # [512, 65536]
    in_maps = []
    for c in range(NCORES):
        in_maps.append({
            "xt": np.ascontiguousarray(xt[:, c * BPC:(c + 1) * BPC]),
            "a": A,
        })
    return in_maps


def gather_out(results):
    y = np.concatenate([r["y"] for r in results], axis=1)      # [512, B]
    return np.ascontiguousarray(y.T.astype(np.float32))


def kernel(x, vectors):
    nc = _get_nc()
    in_maps = prepare_in_maps(x, vectors)
    res = run_bass_kernel_spmd(nc, in_maps, list(range(NCORES)))
    return gather_out(res.results)


if __name__ == "__main__":
    rng = np.random.default_rng(0)
    x = rng.standard_normal((B, S)).astype(np.float32)
    v = rng.standard_normal((NV, S, 1)).astype(np.float32)
    v /= np.linalg.norm(v, axis=1, keepdims=True)
    y = kernel(x, v)
    print("y", y.shape, y.dtype, float(np.abs(y).max()))


# revision 23
# speedup vs baseline: 1.1930x; 1.0146x over previous
"""Trainium2 Bass kernel for the Householder-chain problem.

Computes y = x @ Q.T where Q = M_0 @ M_1 @ ... @ M_{N-1} is a product of
N=514 Householder reflections M_i = I - 2 v_i v_i^T / (v_i^T v_i + eps)
over S=512 dims, and x is [65536, 512].

Since each M_i is symmetric, Q.T = M_{N-1} @ ... @ M_0 =: A, and the
product collapses via the compact-WY representation with natural column
order:  A = I - V T V^T  where V = [v_0 ... v_{N-1}] (S x N) and
T^{-1} = R = stril(V^T V) + diag((||v_i||^2 + eps)/2)   (lower triangular).

Sharding (per the hint: "replicate the small vectors/Q params on all
devices; shard x row-wise"): A is a tiny parameter transformation
(512x512, from the 1.25 MB `vectors` parameter), computed once on the
host in float64 (exact) and replicated to all 8 cores as bf16; x is
sharded row-wise, 8192 rows per core.

The device kernel is the memory-bound streaming matmul y = x @ A, all
bf16: x is transposed + cast to bf16 on the host (halves HBM read
traffic), y is written bf16 and upcast on the host (halves write
traffic).  End-to-end rel err ~2.9e-3 (bf16 rounding of x, A, y;
validated in numpy against the float64 reference), vs the 2e-2 gate.

Device timeline per core: the whole 8 MiB x shard streams in on the SP
DMA queue from t=0 (it fits in SBUF); A arrives on the Activation queue;
the PE then runs 256 back-to-back [128x128]x[128x512] bf16 matmuls
(~71 us, the measured PE streaming rate); PSUM->SBUF bf16 casts
alternate between the Vector and Scalar engines, and y tiles go out in
512-row batches on the Activation DMA queue, overlapped with compute.
"""

from contextlib import ExitStack

import numpy as np
import ml_dtypes

import bass_rust
import concourse.bass as bass
import concourse.mybir as mybir
import concourse.tile as tile
from concourse.bass_utils import run_bass_kernel_spmd
from concourse.vector_clock import ScopedClock

FP = mybir.dt.float32
BF = mybir.dt.bfloat16

S = 512           # feature dim
NV = 514          # number of householder vectors
B = 65536         # batch rows
NCORES = 8
BPC = B // NCORES  # 8192 rows per core
EPS = 1e-16
# main-loop x chunk widths (batch cols per chunk): a small leading chunk so
# the first matmul can start right after the fixed startup; big trailing
# chunks to minimize DMA descriptor count (the DMA queue processes ~135
# descriptors/us, one per partition per transfer).
CHUNKS = [512, 512] + [1024] * 7
assert sum(CHUNKS) == BPC
# y DMA batch widths (batch cols per DMA), per s'-block pass: big batches
# early (4 KB descriptors), small ones at the end so the tail drains fast.
YBATCH = [1024, 1024, 2048, 2048, 1024, 512, 512]
assert sum(YBATCH) == BPC
WARMUP_MM = 10    # dummy matmuls to ramp the PE p-state during DMA


# ---------------------------------------------------------------------------
# walrus CTRL instructions accept at most 4 sem waits, and this Tile
# version puts the whole global-clock wait set on the single tail drain.
# Spread the waits over preceding SP nops (1 wait each, conservatively).
def _patched_drain_and_barrier(self, tick_clock, wait_clock):
    # Leave all the global-clock waits on the drain; _split_excess_waits
    # (run at the end of build_program) hoists the excess onto exactly
    # len(waits)-1 same-engine NOPs.
    drain_inst = self.nc.sync.drain()
    wait_clock.add_sem_waits(
        drain_inst.ins, ScopedClock({None: tick_clock.global_clock})
    )
    self.nc.all_engine_barrier()
    assert self.sems is not None
    popped = self.nc._tile_sem_poison_stack.pop()
    assert popped is self._sem_poison
    # Skip the end-of-program semaphore clear + second barrier (~5 us of
    # teardown): the startup sequence re-initializes every semaphore, so a
    # re-execution of the NEFF is unaffected.  Keep the allocator
    # bookkeeping (mirrors clear_and_free_semaphores minus the emitted
    # instructions) so pool release stays consistent.
    sem_nums = [s.num if hasattr(s, "num") else s
                for s in self.sems.allocated().values()]
    self.nc._state.prepend_free_semaphores(sem_nums)
    for poison_set in self.nc._tile_sem_poison_stack:
        poison_set.update(sem_nums)


tile.TileContext._drain_and_barrier = _patched_drain_and_barrier


def _split_excess_waits(nc, max_waits=1):
    """This walrus build accepts very few sem waits per instruction (a
    TensorTensor with 2 was rejected).  Hoist all but `max_waits` of each
    instruction's waits onto same-engine NOPs inserted right before it —
    engines execute in order, so semantics are unchanged."""
    idx = 0
    for fn in nc.m.functions:
        for bb in fn.blocks:
            new = []
            changed = False
            for inst in bb.instructions:
                si = inst.sync_info
                waits = list(si.on_wait) if si is not None and si.on_wait else []
                if len(waits) > max_waits:
                    changed = True
                    for w in waits[:-max_waits]:
                        idx += 1
                        nop = mybir.InstNoOp(
                            name=f"I-waitsplit-{idx}", engine=inst.engine)
                        nop.sync_info = bass_rust.SyncInfo(
                            on_wait=[w], on_update=[])
                        new.append(nop)
                    upd = list(si.on_update) if si.on_update else []
                    inst.sync_info = bass_rust.SyncInfo(
                        on_wait=waits[-max_waits:], on_update=upd)
                new.append(inst)
            if changed:
                bb.instructions = new
# ---------------------------------------------------------------------------


def build_program(trace_sim=False):
    nc = bass.Bass("TRN2")
    xt_d = nc.dram_tensor("xt", [S, BPC], BF, kind="ExternalInput")
    a_d = nc.dram_tensor("a", [S, S], BF, kind="ExternalInput")
    y_d = nc.dram_tensor("y", [S, BPC], BF, kind="ExternalOutput")

    with tile.TileContext(nc, trace_sim=trace_sim) as tc, ExitStack() as ctx:
        consts = ctx.enter_context(tc.tile_pool(name="consts", bufs=1))
        xbuf = ctx.enter_context(tc.tile_pool(name="xbuf", bufs=1))
        ypool = ctx.enter_context(tc.tile_pool(name="ypool", bufs=4))
        psum_y = ctx.enter_context(
            tc.tile_pool(name="psum_y", bufs=8, space="PSUM"))

        # x prefetch: the whole 8 MiB shard, SP HWDGE queue, from t=0.
        # bc_of[i] = (tiles, col offset within chunk) for batch-col step i
        # (512 cols per step).
        bc_of = []
        col0 = 0
        for c, cw in enumerate(CHUNKS):
            tiles = []
            for k in range(4):
                t = xbuf.tile([128, cw], BF, tag=f"xc{c}_{k}",
                              name=f"xc{c}_{k}")
                nc.sync.dma_start(
                    out=t, in_=xt_d[k * 128:(k + 1) * 128, col0:col0 + cw])
                tiles.append(t)
            for bt in range(cw // 512):
                bc_of.append((tiles, bt * 512))
            col0 += cw

        # A tiles on the Activation HWDGE queue.
        a_bf = []
        for k in range(4):
            t = consts.tile([128, S], BF, tag=f"a{k}", name=f"a{k}")
            nc.scalar.dma_start(out=t, in_=a_d[k * 128:(k + 1) * 128, :])
            a_bf.append(t)

        # PE p-state warmup: harmless matmuls while the DMAs land.
        zs = consts.tile([128, 128], BF, tag="zs", name="zs")
        nc.gpsimd.memset(zs, 0.0)
        zw = consts.tile([128, S], BF, tag="zw", name="zw")
        nc.gpsimd.memset(zw, 0.0)
        for i in range(WARMUP_MM):
            wp = psum_y.tile([128, S], FP, tag="y_ps", name=f"warmup{i}")
            nc.tensor.matmul(wp, lhsT=zs, rhs=zw, start=True, stop=True)

        # main loop over batch-column groups; within a group, one pass per
        # 128-wide s'-block of A, writing yT[st-block, group-cols].  The
        # output is transposed (yT [S, BPC]) so every y DMA is one
        # contiguous multi-KB descriptor per partition, and group-outer
        # order keeps x consumption behind the x DMA stream.
        g0 = 0
        eng = 0
        for bw in YBATCH:
            nb = bw // 512
            for st in range(4):
                ss = slice(st * 128, (st + 1) * 128)
                yt = ypool.tile([128, max(YBATCH)], BF, tag=f"yt{st}", bufs=2)
                for j in range(nb):
                    bc = g0 + j * 512
                    tiles, coff = bc_of[bc // 512]
                    y_ps = psum_y.tile([128, S], FP, tag="y_ps")
                    for k in range(4):
                        nc.tensor.matmul(
                            y_ps, lhsT=a_bf[k][:, ss],
                            rhs=tiles[k][:, coff:coff + 512],
                            start=(k == 0), stop=(k == 3))
                    dst = yt[:, j * S:(j + 1) * S]
                    eng += 1
                    if eng % 2 == 0:
                        nc.vector.tensor_copy(dst, y_ps)
                    else:
                        nc.scalar.copy(dst, y_ps)
                # the last two (small) groups alternate queues: the SP
                # queue's x stream is long done, so splitting the final
                # DMAs across both queues halves the drain tail.
                if bw <= 512 and st % 2 == 0:
                    nc.sync.dma_start(out=y_d[ss, g0:g0 + bw],
                                      in_=yt[:, 0:bw])
                else:
                    nc.scalar.dma_start(out=y_d[ss, g0:g0 + bw],
                                        in_=yt[:, 0:bw])
            g0 += bw
    _split_excess_waits(nc)
    return nc


_NC_CACHE = {}


def _get_nc():
    if "nc" not in _NC_CACHE:
        _NC_CACHE["nc"] = build_program()
    return _NC_CACHE["nc"]


def _compute_A(vectors):
    """Exact (float64) WY collapse of the Householder chain: A = Q^T."""
    v = np.asarray(vectors, dtype=np.float64)[..., 0]   # [514, 512]
    n, s = v.shape
    G = v @ v.T
    d = (np.sum(v * v, axis=1) + EPS) / 2.0
    R = np.tril(G, -1) + np.diag(d)
    X = np.linalg.inv(R)                                # T = R^{-1}
    A = np.eye(s) - v.T @ (X @ v)
    return A


def prepare_in_maps(x, vectors):
    x = np.asarray(x, dtype=np.float32)
    A = _compute_A(vectors).astype(ml_dtypes.bfloat16)  # [512, 512]
    xt = np.ascontiguousarray(x.T).astype(ml_dtypes.bfloat16)  # [512, 65536]
    in_maps = []
    for c in range(NCORES):
        in_maps.append({
            "xt": np.ascontiguousarray(xt[:, c * BPC:(c + 1) * BPC]),
            "a": A,
        })
    return in_maps


def gather_out(results):
    y = np.concatenate([r["y"] for r in results], axis=1)      # [512, B]
    return np.ascontiguousarray(y.T.astype(np.float32))


def kernel(x, vectors):
    nc = _get_nc()
    in_maps = prepare_in_maps(x, vectors)
    res = run_bass_kernel_spmd(nc, in_maps, list(range(NCORES)))
    return gather_out(res.results)


if __name__ == "__main__":
    rng = np.random.default_rng(0)
    x = rng.standard_normal((B, S)).astype(np.float32)
    v = rng.standard_normal((NV, S, 1)).astype(np.float32)
    v /= np.linalg.norm(v, axis=1, keepdims=True)
    y = kernel(x, v)
    print("y", y.shape, y.dtype, float(np.abs(y).max()))
